# revision 5
# baseline (speedup 1.0000x reference)
"""Trainium2 Bass kernel for nn_AttentionLayer (sliding-window attention).

Reference computation (per timestep t, batch b):
    scores = tanh(x @ W) @ proj                  # [T, B]
    for t >= w:  out[t] = sum_j softmax_j(scores[t-w .. t-1]) * x[t-w+j]
    for t <  w:  out[t] = x[t]
with T=2048, B=16, H=1024, w=3.

Strategy (8 NeuronCores, data-parallel over B — 2 batch columns per core):
  Per core, rows r = t*2 + beta (4096 rows of H=1024).  Row-shift of one
  timestep == shift of 2 rows, so out[r] = sum_j a_j[r] * x[r-6+2j].

  1. scores: y^T = W^T-stationary matmuls against host-pretransposed xT
     tiles (PE), tanh on ACT, then s^T = proj^T @ tanh(y^T) on PE (k=1..128
     reduction).  s lands as a [1, 4096] row in SBUF.
  2. softmax over the width-3 window: pure free-dim shifted views of s.
     e_j = exp(s shifted), esum, reciprocal, a_j = e_j * rinv  (ACT + DVE).
  3. weighted sum: one PE matmul per 122-row output tile:
     out = D^T @ x_tile where D is a [128, 122] matrix with the three a_j
     diagonals (built by PE partition-broadcast + DVE copy_predicated with
     constant masks).  The 6-row halo is folded into the x-tile DMA.

kernel() is self-contained: takes full inputs, shards over batch, runs
SPMD on cores 0..7, reassembles the full [T, B, H] output.
"""

import numpy as np

import concourse.bacc as bacc
import concourse.mybir as mybir
import concourse.tile as tile
from concourse.bass_utils import run_bass_kernel_spmd

T_FULL, B_FULL, H = 2048, 16, 1024
N_CORES = 8
BL = B_FULL // N_CORES          # batch columns per core (2)
WWIN = 3                        # attention width (hardcoded)
HALO = WWIN * BL                # 6 rows of halo
MT = 128 - HALO                 # 122 output rows per weighted-sum tile
ST = 512                        # score super-tile rows
KC = H // 128                   # 8 contraction chunks
F32 = mybir.dt.float32
U8 = mybir.dt.uint8
F32R = mybir.dt.float32r

# float32r runs the PE at 1 cycle/col (vs 4 for true fp32) when the moving
# dim is >= 256, at reduced multiply precision.  Score path tolerance is
# loose (feeds a width-3 softmax); the weighted-sum path writes the output.
SCORE_F32R = True
WSUM_F32R = False

TRACE = False                   # set True (from test.py) to capture an NTFF trace
LAST_RESULT = None              # BassKernelResults of the most recent run


def build_nc(R):
    """Build the single-core Bass program for R rows (R % 512 == 0)."""
    assert R % ST == 0
    nst = R // ST
    nmt = (R + MT - 1) // MT

    nc = bacc.Bacc("TRN2", target_bir_lowering=False)

    x_d = nc.dram_tensor("x", [R, H], F32, kind="ExternalInput")
    xT_d = nc.dram_tensor("xT", [H, R], F32, kind="ExternalInput")
    w_d = nc.dram_tensor("w", [H, H], F32, kind="ExternalInput")
    proj_d = nc.dram_tensor("proj", [H], F32, kind="ExternalInput")
    masks_d = nc.dram_tensor("masks", [128, 3 * MT], U8, kind="ExternalInput")
    masks0_d = nc.dram_tensor("masks0", [128, 3 * MT], U8, kind="ExternalInput")
    d0_d = nc.dram_tensor("d0init", [128, MT], F32, kind="ExternalInput")
    out_d = nc.dram_tensor("out", [R, H], F32, kind="ExternalOutput")

    with tile.TileContext(nc) as tc:
        with (
            tc.tile_pool(name="const", bufs=1) as const,
            tc.tile_pool(name="psum_y", bufs=2, space="PSUM") as psum_y,
            tc.tile_pool(name="psum_s", bufs=1, space="PSUM") as psum_s,
            tc.tile_pool(name="psum_bc", bufs=2, space="PSUM") as psum_bc,
            tc.tile_pool(name="psum_o", bufs=3, space="PSUM") as psum_o,
            tc.tile_pool(name="xt", bufs=12) as xt_pool,
            tc.tile_pool(name="th", bufs=4) as th_pool,
            tc.tile_pool(name="xn", bufs=3) as xn_pool,
            tc.tile_pool(name="ob", bufs=3) as ob_pool,
            tc.tile_pool(name="sm", bufs=2) as sm_pool,
        ):
            # ---- constants / persistent buffers ----
            w_sb = const.tile([128, KC * H], F32, name="w_sb")
            nc.sync.dma_start(
                w_sb[:, :].rearrange("p (kc h) -> p kc h", kc=KC),
                w_d.rearrange("(kc p) h -> p kc h", p=128),
            )
            proj_sb = const.tile([128, KC], F32, name="proj_sb")
            nc.sync.dma_start(proj_sb[:, :], proj_d.rearrange("(c p) -> p c", p=128))
            masks_sb = const.tile([128, 3 * MT], U8, name="masks_sb")
            nc.sync.dma_start(masks_sb[:, :], masks_d[:, :])
            masks0_sb = const.tile([128, 3 * MT], U8, name="masks0_sb")
            nc.sync.dma_start(masks0_sb[:, :], masks0_d[:, :])
            d0t = const.tile([128, MT], F32, name="d0t")
            nc.sync.dma_start(d0t[:, :], d0_d[:, :])
            dA = const.tile([128, MT], F32, name="dA")
            dB = const.tile([128, MT], F32, name="dB")
            nc.vector.memset(dA[:, :], 0.0)
            nc.vector.memset(dB[:, :], 0.0)
            ones_sb = const.tile([1, 128], F32, name="ones_sb")
            nc.vector.memset(ones_sb[:, :], 1.0)
            # s_buf[0, 6 + r] = scores[r]; zero prefix covers the r < 6 window
            s_buf = const.tile([1, HALO + R + 64], F32, name="s_buf")
            nc.vector.memset(s_buf[:, :], 0.0)
            # e_j holds exp(s[r - 6 + 2j]) then, in place, the softmax weight a_j[r]
            e_bufs = [const.tile([1, R], F32, name=f"e{j}") for j in range(WWIN)]

            def emit_score(st):
                r0 = st * ST
                xts = []
                for kc in range(KC):
                    xt_t = xt_pool.tile([128, ST], F32, name="xt_t", tag="xt")
                    nc.sync.dma_start(
                        xt_t[:, :], xT_d[kc * 128:(kc + 1) * 128, r0:r0 + ST]
                    )
                    xts.append(xt_t)
                ths = []
                for ho in range(KC):
                    ypsum = psum_y.tile([128, ST], F32, name="ypsum", tag="ypsum")
                    for kc in range(KC):
                        off = kc * H + ho * 128
                        lw = w_sb[:, off:off + 128]
                        rx = xts[kc][:, :]
                        if SCORE_F32R:
                            lw, rx = lw.bitcast(F32R), rx.bitcast(F32R)
                        nc.tensor.matmul(
                            ypsum[:, :],
                            lhsT=lw,
                            rhs=rx,
                            start=(kc == 0),
                            stop=(kc == KC - 1),
                        )
                    th = th_pool.tile([128, ST], F32, name="th", tag="th")
                    nc.scalar.activation(
                        th[:, :], ypsum[:, :], mybir.ActivationFunctionType.Tanh
                    )
                    ths.append(th)
                spsum = psum_s.tile([1, ST], F32, name="spsum", tag="spsum")
                for ho in range(KC):
                    lp = proj_sb[:, ho:ho + 1]
                    rt = ths[ho][:, :]
                    if SCORE_F32R:
                        lp, rt = lp.bitcast(F32R), rt.bitcast(F32R)
                    nc.tensor.matmul(
                        spsum[:, :],
                        lhsT=lp,
                        rhs=rt,
                        start=(ho == 0),
                        stop=(ho == KC - 1),
                    )
                nc.scalar.copy(s_buf[0:1, HALO + r0:HALO + r0 + ST], spsum[:, :])
                for j in range(WWIN):
                    nc.scalar.activation(
                        e_bufs[j][0:1, r0:r0 + ST],
                        s_buf[0:1, r0 + 2 * j:r0 + 2 * j + ST],
                        mybir.ActivationFunctionType.Exp,
                    )
                esum = sm_pool.tile([1, ST], F32, name="esum", tag="esum")
                nc.vector.tensor_add(
                    esum[:, :], e_bufs[0][0:1, r0:r0 + ST], e_bufs[1][0:1, r0:r0 + ST]
                )
                nc.vector.tensor_add(
                    esum[:, :], esum[:, :], e_bufs[2][0:1, r0:r0 + ST]
                )
                nc.vector.reciprocal(esum[:, :], esum[:, :])
                for j in range(WWIN):
                    nc.vector.tensor_mul(
                        e_bufs[j][0:1, r0:r0 + ST],
                        e_bufs[j][0:1, r0:r0 + ST],
                        esum[:, :],
                    )

            def emit_wsum(mt):
                m0 = mt * MT
                m_n = min(MT, R - m0)
                if mt == 0:
                    src0, k_n, dmat, msk = 0, 128, d0t, masks0_sb
                else:
                    src0 = m0 - HALO
                    k_n = min(128, R - src0)
                    dmat = dA if (mt % 2) else dB
                    msk = masks_sb
                xn = xn_pool.tile([128, H], F32, name="xn", tag="xn")
                nc.sync.dma_start(xn[0:k_n, :], x_d[src0:src0 + k_n, :])
                for j in range(WWIN):
                    bc = psum_bc.tile([128, MT], F32, name="bc", tag="bc")
                    nc.tensor.matmul(
                        bc[:, 0:m_n],
                        lhsT=ones_sb[:, :],
                        rhs=e_bufs[j][0:1, m0:m0 + m_n],
                        start=True,
                        stop=True,
                    )
                    nc.vector.copy_predicated(
                        dmat[:, 0:m_n], msk[:, j * MT:j * MT + m_n], bc[:, 0:m_n]
                    )
                ob = ob_pool.tile([MT, H], F32, name="ob", tag="ob")
                for half in range(2):
                    op_ = psum_o.tile([MT, 512], F32, name="op_", tag="opsum")
                    ld = dmat[0:k_n, 0:m_n]
                    rn = xn[0:k_n, half * 512:(half + 1) * 512]
                    if WSUM_F32R:
                        ld, rn = ld.bitcast(F32R), rn.bitcast(F32R)
                    nc.tensor.matmul(
                        op_[0:m_n, :],
                        lhsT=ld,
                        rhs=rn,
                        start=True,
                        stop=True,
                    )
                    nc.vector.tensor_copy(
                        ob[0:m_n, half * 512:(half + 1) * 512], op_[0:m_n, :]
                    )
                nc.sync.dma_start(out_d[m0:m0 + m_n, :], ob[0:m_n, :])

            # software-pipelined emission: weighted-sum tiles are emitted as
            # soon as the score super-tile covering their rows is emitted
            ready = {st: [] for st in range(nst)}
            for mt in range(nmt):
                last_row = min(mt * MT + min(MT, R - mt * MT) - 1, R - 1)
                ready[min(last_row // ST, nst - 1)].append(mt)
            for st in range(nst):
                emit_score(st)
                for mt in ready[st]:
                    emit_wsum(mt)

    nc.compile()
    return nc


def make_consts():
    masks = np.zeros((128, 3 * MT), np.uint8)
    masks0 = np.zeros((128, 3 * MT), np.uint8)
    d0 = np.zeros((128, MT), np.float32)
    for j in range(WWIN):
        for m in range(MT):
            masks[m + 2 * j, j * MT + m] = 1
            if m >= HALO:
                masks0[m - HALO + 2 * j, j * MT + m] = 1
    for m in range(HALO):
        d0[m, m] = 1.0
    return masks, masks0, d0


_NC_CACHE = {}


def _get_nc(R):
    if R not in _NC_CACHE:
        _NC_CACHE[R] = build_nc(R)
    return _NC_CACHE[R]


def make_in_maps(x, weight_W, weight_proj):
    """x: [T, B, H] fp32 -> list of per-core input dicts."""
    W = np.ascontiguousarray(np.asarray(weight_W, dtype=np.float32))
    proj = np.ascontiguousarray(
        np.asarray(weight_proj, dtype=np.float32).reshape(H)
    )
    masks, masks0, d0 = make_consts()
    t = x.shape[0]
    in_maps = []
    for c in range(N_CORES):
        xc = np.ascontiguousarray(x[:, BL * c:BL * (c + 1), :]).reshape(t * BL, H)
        in_maps.append(
            dict(
                x=xc,
                xT=np.ascontiguousarray(xc.T),
                w=W,
                proj=proj,
                masks=masks,
                masks0=masks0,
                d0init=d0,
            )
        )
    return in_maps


def kernel(inputs, weight_W, weight_proj, attention_width):
    global LAST_RESULT
    assert int(attention_width) == WWIN
    x = np.ascontiguousarray(np.asarray(inputs, dtype=np.float32))
    t, b, h = x.shape
    assert b == B_FULL and h == H
    r = t * BL
    in_maps = make_in_maps(x, weight_W, weight_proj)
    nc = _get_nc(r)
    res = run_bass_kernel_spmd(
        nc, in_maps, core_ids=list(range(N_CORES)), trace=TRACE
    )
    LAST_RESULT = res
    out = np.empty((t, B_FULL, H), np.float32)
    for c, rmap in enumerate(res.results):
        out[:, BL * c:BL * (c + 1), :] = rmap["out"].reshape(t, BL, H)
    return out


# revision 6
# speedup vs baseline: 1.0560x; 1.0560x over previous
"""Trainium2 Bass kernel for nn_AttentionLayer (sliding-window attention).

Reference computation (per timestep t, batch b):
    scores = tanh(x @ W) @ proj                  # [T, B]
    for t >= w:  out[t] = sum_j softmax_j(scores[t-w .. t-1]) * x[t-w+j]
    for t <  w:  out[t] = x[t]
with T=2048, B=16, H=1024, w=3.

Strategy (8 NeuronCores, data-parallel over B — 2 batch columns per core):
  Per core, rows r = t*2 + beta (4096 rows of H=1024).  Row-shift of one
  timestep == shift of 2 rows, so out[r] = sum_j a_j[r] * x[r-6+2j].

  1. scores: y^T = W^T-stationary matmuls against host-pretransposed xT
     tiles (PE), tanh on ACT, then s^T = proj^T @ tanh(y^T) on PE (k=1..128
     reduction).  s lands as a [1, 4096] row in SBUF.
  2. softmax over the width-3 window: pure free-dim shifted views of s.
     e_j = exp(s shifted), esum, reciprocal, a_j = e_j * rinv  (ACT + DVE).
  3. weighted sum: one PE matmul per 122-row output tile:
     out = D^T @ x_tile where D is a [128, 122] matrix with the three a_j
     diagonals (built by PE partition-broadcast + DVE copy_predicated with
     constant masks).  The 6-row halo is folded into the x-tile DMA.

kernel() is self-contained: takes full inputs, shards over batch, runs
SPMD on cores 0..7, reassembles the full [T, B, H] output.
"""

import numpy as np

import concourse.bacc as bacc
import concourse.mybir as mybir
import concourse.tile as tile
from concourse.bass_utils import run_bass_kernel_spmd

T_FULL, B_FULL, H = 2048, 16, 1024
N_CORES = 8
BL = B_FULL // N_CORES          # batch columns per core (2)
WWIN = 3                        # attention width (hardcoded)
HALO = WWIN * BL                # 6 rows of halo
MT = 128 - HALO                 # 122 output rows per weighted-sum tile
ST = 512                        # score super-tile rows
KC = H // 128                   # 8 contraction chunks
F32 = mybir.dt.float32
U8 = mybir.dt.uint8
F32R = mybir.dt.float32r

# float32r runs the PE at 1 cycle/col (vs 4 for true fp32) when the moving
# dim is >= 256, at reduced multiply precision.  Score path tolerance is
# loose (feeds a width-3 softmax); the weighted-sum path writes the output.
SCORE_F32R = True
WSUM_F32R = False

TRACE = False                   # set True (from test.py) to capture an NTFF trace
LAST_RESULT = None              # BassKernelResults of the most recent run


def build_nc(R):
    """Build the single-core Bass program for R rows (R % 512 == 0)."""
    assert R % ST == 0
    nst = R // ST
    nmt = (R + MT - 1) // MT

    # float32r tensors carry float32 bits; the PE runs them single-pass
    # (4x faster than true fp32) at reduced multiply precision.
    sd = F32R if SCORE_F32R else F32     # score path (xT, W, proj, tanh out)
    wd = F32R if WSUM_F32R else F32      # weighted-sum path (x, D)

    nc = bacc.Bacc("TRN2", target_bir_lowering=False)

    x_d = nc.dram_tensor("x", [R, H], wd, kind="ExternalInput")
    xT_d = nc.dram_tensor("xT", [H, R], sd, kind="ExternalInput")
    w_d = nc.dram_tensor("w", [H, H], sd, kind="ExternalInput")
    proj_d = nc.dram_tensor("proj", [H], sd, kind="ExternalInput")
    masks_d = nc.dram_tensor("masks", [128, 3 * MT], U8, kind="ExternalInput")
    masks0_d = nc.dram_tensor("masks0", [128, 3 * MT], U8, kind="ExternalInput")
    d0_d = nc.dram_tensor("d0init", [128, MT], wd, kind="ExternalInput")
    out_d = nc.dram_tensor("out", [R, H], F32, kind="ExternalOutput")

    with tile.TileContext(nc) as tc:
        with (
            tc.tile_pool(name="const", bufs=1) as const,
            tc.tile_pool(name="psum_y", bufs=2, space="PSUM") as psum_y,
            tc.tile_pool(name="psum_s", bufs=1, space="PSUM") as psum_s,
            tc.tile_pool(name="psum_bc", bufs=2, space="PSUM") as psum_bc,
            tc.tile_pool(name="psum_o", bufs=3, space="PSUM") as psum_o,
            tc.tile_pool(name="xt", bufs=12) as xt_pool,
            tc.tile_pool(name="th", bufs=4) as th_pool,
            tc.tile_pool(name="xn", bufs=3) as xn_pool,
            tc.tile_pool(name="ob", bufs=3) as ob_pool,
            tc.tile_pool(name="sm", bufs=2) as sm_pool,
        ):
            # ---- constants / persistent buffers ----
            w_sb = const.tile([128, KC * H], sd, name="w_sb")
            nc.sync.dma_start(
                w_sb[:, :].rearrange("p (kc h) -> p kc h", kc=KC),
                w_d.rearrange("(kc p) h -> p kc h", p=128),
            )
            proj_sb = const.tile([128, KC], sd, name="proj_sb")
            nc.sync.dma_start(proj_sb[:, :], proj_d.rearrange("(c p) -> p c", p=128))
            masks_sb = const.tile([128, 3 * MT], U8, name="masks_sb")
            nc.sync.dma_start(masks_sb[:, :], masks_d[:, :])
            masks0_sb = const.tile([128, 3 * MT], U8, name="masks0_sb")
            nc.sync.dma_start(masks0_sb[:, :], masks0_d[:, :])
            d0t = const.tile([128, MT], wd, name="d0t")
            nc.sync.dma_start(d0t[:, :], d0_d[:, :])
            dA = const.tile([128, MT], wd, name="dA")
            dB = const.tile([128, MT], wd, name="dB")
            nc.vector.memset(dA[:, :], 0.0)
            nc.vector.memset(dB[:, :], 0.0)
            ones_sb = const.tile([1, 128], F32, name="ones_sb")
            nc.vector.memset(ones_sb[:, :], 1.0)
            # s_buf[0, 6 + r] = scores[r]; zero prefix covers the r < 6 window
            s_buf = const.tile([1, HALO + R + 64], F32, name="s_buf")
            nc.vector.memset(s_buf[:, :], 0.0)
            # e_j holds exp(s[r - 6 + 2j]) then, in place, the softmax weight a_j[r]
            e_bufs = [const.tile([1, R], F32, name=f"e{j}") for j in range(WWIN)]

            def emit_score(st):
                r0 = st * ST
                xts = []
                for kc in range(KC):
                    xt_t = xt_pool.tile([128, ST], sd, name="xt_t", tag="xt")
                    nc.sync.dma_start(
                        xt_t[:, :], xT_d[kc * 128:(kc + 1) * 128, r0:r0 + ST]
                    )
                    xts.append(xt_t)
                ths = []
                for ho in range(KC):
                    ypsum = psum_y.tile([128, ST], F32, name="ypsum", tag="ypsum")
                    for kc in range(KC):
                        off = kc * H + ho * 128
                        nc.tensor.matmul(
                            ypsum[:, :],
                            lhsT=w_sb[:, off:off + 128],
                            rhs=xts[kc][:, :],
                            start=(kc == 0),
                            stop=(kc == KC - 1),
                        )
                    th = th_pool.tile([128, ST], sd, name="th", tag="th")
                    nc.scalar.activation(
                        th[:, :], ypsum[:, :], mybir.ActivationFunctionType.Tanh
                    )
                    ths.append(th)
                spsum = psum_s.tile([1, ST], F32, name="spsum", tag="spsum")
                for ho in range(KC):
                    nc.tensor.matmul(
                        spsum[:, :],
                        lhsT=proj_sb[:, ho:ho + 1],
                        rhs=ths[ho][:, :],
                        start=(ho == 0),
                        stop=(ho == KC - 1),
                    )
                nc.scalar.copy(s_buf[0:1, HALO + r0:HALO + r0 + ST], spsum[:, :])
                for j in range(WWIN):
                    nc.scalar.activation(
                        e_bufs[j][0:1, r0:r0 + ST],
                        s_buf[0:1, r0 + 2 * j:r0 + 2 * j + ST],
                        mybir.ActivationFunctionType.Exp,
                    )
                esum = sm_pool.tile([1, ST], F32, name="esum", tag="esum")
                nc.vector.tensor_add(
                    esum[:, :], e_bufs[0][0:1, r0:r0 + ST], e_bufs[1][0:1, r0:r0 + ST]
                )
                nc.vector.tensor_add(
                    esum[:, :], esum[:, :], e_bufs[2][0:1, r0:r0 + ST]
                )
                nc.vector.reciprocal(esum[:, :], esum[:, :])
                for j in range(WWIN):
                    nc.vector.tensor_mul(
                        e_bufs[j][0:1, r0:r0 + ST],
                        e_bufs[j][0:1, r0:r0 + ST],
                        esum[:, :],
                    )

            def emit_wsum(mt):
                m0 = mt * MT
                m_n = min(MT, R - m0)
                if mt == 0:
                    src0, k_n, dmat, msk = 0, 128, d0t, masks0_sb
                else:
                    src0 = m0 - HALO
                    k_n = min(128, R - src0)
                    dmat = dA if (mt % 2) else dB
                    msk = masks_sb
                xn = xn_pool.tile([128, H], wd, name="xn", tag="xn")
                nc.sync.dma_start(xn[0:k_n, :], x_d[src0:src0 + k_n, :])
                for j in range(WWIN):
                    bc = psum_bc.tile([128, MT], F32, name="bc", tag="bc")
                    nc.tensor.matmul(
                        bc[:, 0:m_n],
                        lhsT=ones_sb[:, :],
                        rhs=e_bufs[j][0:1, m0:m0 + m_n],
                        start=True,
                        stop=True,
                    )
                    nc.vector.copy_predicated(
                        dmat[:, 0:m_n], msk[:, j * MT:j * MT + m_n], bc[:, 0:m_n]
                    )
                ob = ob_pool.tile([MT, H], F32, name="ob", tag="ob")
                for half in range(2):
                    op_ = psum_o.tile([MT, 512], F32, name="op_", tag="opsum")
                    nc.tensor.matmul(
                        op_[0:m_n, :],
                        lhsT=dmat[0:k_n, 0:m_n],
                        rhs=xn[0:k_n, half * 512:(half + 1) * 512],
                        start=True,
                        stop=True,
                    )
                    nc.vector.tensor_copy(
                        ob[0:m_n, half * 512:(half + 1) * 512], op_[0:m_n, :]
                    )
                nc.sync.dma_start(out_d[m0:m0 + m_n, :], ob[0:m_n, :])

            # software-pipelined emission: weighted-sum tiles are emitted as
            # soon as the score super-tile covering their rows is emitted
            ready = {st: [] for st in range(nst)}
            for mt in range(nmt):
                last_row = min(mt * MT + min(MT, R - mt * MT) - 1, R - 1)
                ready[min(last_row // ST, nst - 1)].append(mt)
            for st in range(nst):
                emit_score(st)
                for mt in ready[st]:
                    emit_wsum(mt)

    nc.compile()
    return nc


def make_consts():
    masks = np.zeros((128, 3 * MT), np.uint8)
    masks0 = np.zeros((128, 3 * MT), np.uint8)
    d0 = np.zeros((128, MT), np.float32)
    for j in range(WWIN):
        for m in range(MT):
            masks[m + 2 * j, j * MT + m] = 1
            if m >= HALO:
                masks0[m - HALO + 2 * j, j * MT + m] = 1
    for m in range(HALO):
        d0[m, m] = 1.0
    return masks, masks0, d0


_NC_CACHE = {}


def _get_nc(R):
    key = (R, SCORE_F32R, WSUM_F32R)
    if key not in _NC_CACHE:
        _NC_CACHE[key] = build_nc(R)
    return _NC_CACHE[key]


def make_in_maps(x, weight_W, weight_proj):
    """x: [T, B, H] fp32 -> list of per-core input dicts."""
    W = np.ascontiguousarray(np.asarray(weight_W, dtype=np.float32))
    proj = np.ascontiguousarray(
        np.asarray(weight_proj, dtype=np.float32).reshape(H)
    )
    masks, masks0, d0 = make_consts()
    t = x.shape[0]
    in_maps = []
    for c in range(N_CORES):
        xc = np.ascontiguousarray(x[:, BL * c:BL * (c + 1), :]).reshape(t * BL, H)
        in_maps.append(
            dict(
                x=xc,
                xT=np.ascontiguousarray(xc.T),
                w=W,
                proj=proj,
                masks=masks,
                masks0=masks0,
                d0init=d0,
            )
        )
    return in_maps


def kernel(inputs, weight_W, weight_proj, attention_width):
    global LAST_RESULT
    assert int(attention_width) == WWIN
    x = np.ascontiguousarray(np.asarray(inputs, dtype=np.float32))
    t, b, h = x.shape
    assert b == B_FULL and h == H
    r = t * BL
    in_maps = make_in_maps(x, weight_W, weight_proj)
    nc = _get_nc(r)
    res = run_bass_kernel_spmd(
        nc, in_maps, core_ids=list(range(N_CORES)), trace=TRACE
    )
    LAST_RESULT = res
    out = np.empty((t, B_FULL, H), np.float32)
    for c, rmap in enumerate(res.results):
        out[:, BL * c:BL * (c + 1), :] = rmap["out"].reshape(t, BL, H)
    return out


# revision 9
# speedup vs baseline: 1.0986x; 1.0404x over previous
"""Trainium2 Bass kernel for nn_AttentionLayer (sliding-window attention).

Reference computation (per timestep t, batch b):
    scores = tanh(x @ W) @ proj                  # [T, B]
    for t >= w:  out[t] = sum_j softmax_j(scores[t-w .. t-1]) * x[t-w+j]
    for t <  w:  out[t] = x[t]
with T=2048, B=16, H=1024, w=3.

Strategy (8 NeuronCores, data-parallel over B — 2 batch columns per core):
  Per core, rows r = t*2 + beta (4096 rows of H=1024).  Row-shift of one
  timestep == shift of 2 rows, so out[r] = sum_j a_j[r] * x[r-6+2j].

  1. scores: y^T = W-stationary matmuls against host-pretransposed xT tiles
     (PE, bf16 in / fp32 accumulate), tanh on ACT (float32r out), then
     s^T = proj^T @ tanh(y^T) on PE (float32r, single-pass full-rate).
     s lands as a [1, 4096] row in SBUF.
  2. softmax over the width-3 window: pure free-dim shifted views of s.
     One exp pass (ACT), esum + reciprocal + 3 normalized-weight writes
     (DVE, fp16 weights).
  3. weighted sum: one fp32 PE matmul per 122-row output tile:
     out = D^T @ x_tile where D is a [128, 122] fp32 matrix carrying the
     three a_j diagonals.  D is built for four tiles at a time: an fp16
     rank-1 PE broadcast of the weight rows into PSUM, then DVE
     copy_predicated with constant uint8 diagonal masks.  The 6-row halo
     is folded into the x-tile DMA, so no partition-shifted vector ops
     exist anywhere.

kernel() is self-contained: takes full inputs, shards over batch, runs
SPMD on cores 0..7, reassembles the full [T, B, H] output.
"""

import numpy as np
import ml_dtypes

import concourse.bacc as bacc
import concourse.mybir as mybir
import concourse.tile as tile
from concourse.bass_utils import run_bass_kernel_spmd

T_FULL, B_FULL, H = 2048, 16, 1024
N_CORES = 8
BL = B_FULL // N_CORES          # batch columns per core (2)
WWIN = 3                        # attention width (hardcoded)
HALO = WWIN * BL                # 6 rows of halo
MT = 128 - HALO                 # 122 output rows per weighted-sum tile
QT = 4                          # m-tiles per D-build quad group
ST = 512                        # score super-tile rows
KC = H // 128                   # 8 contraction chunks
F32 = mybir.dt.float32
F32R = mybir.dt.float32r
BF16 = mybir.dt.bfloat16
F16 = mybir.dt.float16
U8 = mybir.dt.uint8

TRACE = False                   # set True (from test.py) to capture an NTFF trace
LAST_RESULT = None              # BassKernelResults of the most recent run

# dtype knobs (all measured on HW):
#   SCORE_BF16: xT/W in bf16 (halves score DMA; PE same or faster)
#   WSUM_F16:   D and x in fp16 for the weighted sum (4x PE, ~1e-3 output err)
SCORE_BF16 = False
WSUM_F16 = True


def _mt_span(R, mt):
    m0 = mt * MT
    return m0, min(MT, R - m0)


def build_nc(R):
    """Build the single-core Bass program for R rows (R % 512 == 0)."""
    assert R % ST == 0
    nst = R // ST
    nmt = (R + MT - 1) // MT
    ngr = (nmt + QT - 1) // QT
    QW = QT * MT                 # weight columns per quad group (488)

    sd = BF16 if SCORE_BF16 else F32R     # score matmul input dtype
    wd = F16 if WSUM_F16 else F32         # weighted-sum matmul dtype

    nc = bacc.Bacc("TRN2", target_bir_lowering=False)

    x_d = nc.dram_tensor("x", [R, H], wd, kind="ExternalInput")
    xT_d = nc.dram_tensor("xT", [H, R], sd, kind="ExternalInput")
    w_d = nc.dram_tensor("w", [H, H], sd, kind="ExternalInput")
    proj_d = nc.dram_tensor("proj", [H], F32R, kind="ExternalInput")
    masksq_d = nc.dram_tensor("masksq", [128, 3 * QT * MT], U8, kind="ExternalInput")
    masksq0_d = nc.dram_tensor("masksq0", [128, 3 * QT * MT], U8, kind="ExternalInput")
    d0_d = nc.dram_tensor("d0init", [128, QT * MT], wd, kind="ExternalInput")
    xh_d = nc.dram_tensor("x_head", [8, H], F32, kind="ExternalInput")
    out_d = nc.dram_tensor("out", [R, H], F32, kind="ExternalOutput")
    QW = QT * MT

    with tile.TileContext(nc) as tc:
        with (
            tc.tile_pool(name="const", bufs=1) as const,
            tc.tile_pool(name="psum_y", bufs=2, space="PSUM") as psum_y,
            tc.tile_pool(name="psum_s", bufs=1, space="PSUM") as psum_s,
            tc.tile_pool(name="psum_bc", bufs=2, space="PSUM") as psum_bc,
            tc.tile_pool(name="psum_o", bufs=3, space="PSUM") as psum_o,
            tc.tile_pool(name="xt", bufs=12) as xt_pool,
            tc.tile_pool(name="th", bufs=4) as th_pool,
            tc.tile_pool(name="xn", bufs=3) as xn_pool,
            tc.tile_pool(name="ob", bufs=3) as ob_pool,
            tc.tile_pool(name="sm", bufs=2) as sm_pool,
        ):
            # ---- constants / persistent buffers ----
            w_sb = const.tile([128, KC * H], sd, name="w_sb")
            nc.sync.dma_start(
                w_sb[:, :].rearrange("p (kc h) -> p kc h", kc=KC),
                w_d.rearrange("(kc p) h -> p kc h", p=128),
            )
            proj_sb = const.tile([128, KC], F32R, name="proj_sb")
            nc.sync.dma_start(proj_sb[:, :], proj_d.rearrange("(c p) -> p c", p=128))
            masksq_sb = const.tile([128, 3 * QW], U8, name="masksq_sb")
            nc.sync.dma_start(masksq_sb[:, :], masksq_d[:, :])
            masksq0_sb = const.tile([128, 3 * QW], U8, name="masksq0_sb")
            nc.sync.dma_start(masksq0_sb[:, :], masksq0_d[:, :])
            d0t = const.tile([128, QW], wd, name="d0t")
            nc.sync.dma_start(d0t[:, :], d0_d[:, :])
            dA = const.tile([128, QW], wd, name="dA")
            dB = const.tile([128, QW], wd, name="dB")
            nc.vector.memset(dA[:, :], 0.0)
            nc.vector.memset(dB[:, :], 0.0)
            ones_sb = const.tile([1, 128], F16, name="ones_sb")
            nc.vector.memset(ones_sb[:, :], 1.0)
            # s_buf[0, 6 + r] = scores[r]; zero prefix covers the r < 6 window
            s_buf = const.tile([1, HALO + R + 64], F32, name="s_buf")
            nc.vector.memset(s_buf[:, :], 0.0)
            # e_all[0, i] = exp(s_buf[0, i]); a_j[0, r] = softmax weight (fp16)
            e_all = const.tile([1, R + 8], F32, name="e_all")
            a_bufs = [const.tile([1, R], F16, name=f"a{j}") for j in range(WWIN)]

            def emit_score(st):
                r0 = st * ST
                xts = []
                for kc in range(KC):
                    xt_t = xt_pool.tile([128, ST], sd, name="xt_t", tag="xt")
                    nc.sync.dma_start(
                        xt_t[:, :], xT_d[kc * 128:(kc + 1) * 128, r0:r0 + ST]
                    )
                    xts.append(xt_t)
                ths = []
                for ho in range(KC):
                    ypsum = psum_y.tile([128, ST], F32, name="ypsum", tag="ypsum")
                    for kc in range(KC):
                        off = kc * H + ho * 128
                        nc.tensor.matmul(
                            ypsum[:, :],
                            lhsT=w_sb[:, off:off + 128],
                            rhs=xts[kc][:, :],
                            start=(kc == 0),
                            stop=(kc == KC - 1),
                        )
                    th = th_pool.tile([128, ST], F32R, name="th", tag="th")
                    nc.scalar.activation(
                        th[:, :], ypsum[:, :], mybir.ActivationFunctionType.Tanh
                    )
                    ths.append(th)
                spsum = psum_s.tile([1, ST], F32, name="spsum", tag="spsum")
                for ho in range(KC):
                    nc.tensor.matmul(
                        spsum[:, :],
                        lhsT=proj_sb[:, ho:ho + 1],
                        rhs=ths[ho][:, :],
                        start=(ho == 0),
                        stop=(ho == KC - 1),
                    )
                nc.scalar.copy(s_buf[0:1, HALO + r0:HALO + r0 + ST], spsum[:, :])
                # one exp pass over this chunk of s (overlapping 4 into the
                # next chunk so the window sums below never read ahead)
                en = min(ST + 4, R + 4 - r0)
                nc.scalar.activation(
                    e_all[0:1, r0:r0 + en],
                    s_buf[0:1, r0:r0 + en],
                    mybir.ActivationFunctionType.Exp,
                )
                esum = sm_pool.tile([1, ST], F32, name="esum", tag="esum")
                nc.vector.tensor_add(
                    esum[:, :], e_all[0:1, r0:r0 + ST], e_all[0:1, r0 + 2:r0 + 2 + ST]
                )
                nc.vector.tensor_add(
                    esum[:, :], esum[:, :], e_all[0:1, r0 + 4:r0 + 4 + ST]
                )
                nc.vector.reciprocal(esum[:, :], esum[:, :])
                for j in range(WWIN):
                    nc.vector.tensor_mul(
                        a_bufs[j][0:1, r0:r0 + ST],
                        e_all[0:1, r0 + 2 * j:r0 + 2 * j + ST],
                        esum[:, :],
                    )

            def emit_group(g):
                """Build the quad D matrix for m-tiles [QT*g, QT*g+QT) and run
                their weighted-sum matmuls."""
                mts = list(range(QT * g, min(QT * g + QT, nmt)))
                g0 = QT * g * MT                            # first weight column
                gn = sum(_mt_span(R, mt)[1] for mt in mts)  # total weight cols
                if g == 0:
                    dmat, msk = d0t, masksq0_sb
                else:
                    dmat = dA if (g % 2) else dB
                    msk = masksq_sb
                for j in range(WWIN):
                    bcast = psum_bc.tile([128, QW], F32, name="bcast", tag="bc")
                    nc.tensor.matmul(
                        bcast[:, 0:gn],
                        lhsT=ones_sb[:, :],
                        rhs=a_bufs[j][0:1, g0:g0 + gn],
                        start=True,
                        stop=True,
                    )
                    nc.vector.copy_predicated(
                        dmat[:, 0:gn], msk[:, j * QW:j * QW + gn], bcast[:, 0:gn]
                    )
                for qi, mt in enumerate(mts):
                    m0, m_n = _mt_span(R, mt)
                    if mt == 0:
                        src0, k_n = 0, 128
                    else:
                        src0 = m0 - HALO
                        k_n = min(128, R - src0)
                    xn = xn_pool.tile([128, H], wd, name="xn", tag="xn")
                    nc.sync.dma_start(xn[0:k_n, :], x_d[src0:src0 + k_n, :])
                    ob = ob_pool.tile([MT, H], F32, name="ob", tag="ob")
                    for half in range(2):
                        op_ = psum_o.tile([MT, 512], F32, name="op_", tag="opsum")
                        nc.tensor.matmul(
                            op_[0:m_n, :],
                            lhsT=dmat[0:k_n, qi * MT:qi * MT + m_n],
                            rhs=xn[0:k_n, half * 512:(half + 1) * 512],
                            start=True,
                            stop=True,
                        )
                        if half == 0:
                            nc.vector.tensor_copy(ob[0:m_n, 0:512], op_[0:m_n, :])
                        else:
                            nc.scalar.copy(ob[0:m_n, 512:1024], op_[0:m_n, :])
                    if mt == 0:
                        # exact fp32 passthrough for rows with t < w
                        nc.sync.dma_start(ob[0:HALO, :], xh_d[0:HALO, :])
                    nc.sync.dma_start(out_d[m0:m0 + m_n, :], ob[0:m_n, :])

            # software-pipelined emission: a quad group is emitted as soon as
            # the score super-tile covering its last row has been emitted
            ready = {st: [] for st in range(nst)}
            for g in range(ngr):
                last_mt = min(QT * g + QT, nmt) - 1
                last_row = min(last_mt * MT + _mt_span(R, last_mt)[1] - 1, R - 1)
                ready[min(last_row // ST, nst - 1)].append(g)
            for st in range(nst):
                emit_score(st)
                for g in ready[st]:
                    emit_group(g)

    nc.compile()
    return nc


def make_consts(R):
    QW = QT * MT
    masksq = np.zeros((128, 3 * QW), np.uint8)
    masksq0 = np.zeros((128, 3 * QW), np.uint8)
    d0 = np.zeros((128, QW), np.float32)
    for j in range(WWIN):
        for q in range(QT):
            for m in range(MT):
                col = j * QW + q * MT + m
                masksq[m + 2 * j, col] = 1
                if q == 0:
                    if m >= HALO:
                        masksq0[m - HALO + 2 * j, col] = 1
                else:
                    masksq0[m + 2 * j, col] = 1
    for m in range(HALO):
        d0[m, m] = 1.0
    return masksq, masksq0, d0


_NC_CACHE = {}


def _get_nc(R):
    key = (R, SCORE_BF16, WSUM_F16)
    if key not in _NC_CACHE:
        _NC_CACHE[key] = build_nc(R)
    return _NC_CACHE[key]


def make_in_maps(x, weight_W, weight_proj):
    """x: [T, B, H] fp32 -> list of per-core input dicts."""
    sd = ml_dtypes.bfloat16 if SCORE_BF16 else np.float32
    wd = np.float16 if WSUM_F16 else np.float32
    W = np.ascontiguousarray(np.asarray(weight_W, dtype=np.float32)).astype(sd)
    proj = np.ascontiguousarray(
        np.asarray(weight_proj, dtype=np.float32).reshape(H)
    )
    t = x.shape[0]
    masksq, masksq0, d0 = make_consts(t * BL)
    d0 = d0.astype(wd)
    in_maps = []
    for c in range(N_CORES):
        xc = np.ascontiguousarray(x[:, BL * c:BL * (c + 1), :]).reshape(t * BL, H)
        in_maps.append(
            dict(
                x_head=np.ascontiguousarray(xc[0:8]),
                x=xc.astype(wd) if wd is not np.float32 else xc,
                xT=np.ascontiguousarray(xc.T).astype(sd),
                w=W,
                proj=proj,
                masksq=masksq,
                masksq0=masksq0,
                d0init=d0,
            )
        )
    return in_maps


def kernel(inputs, weight_W, weight_proj, attention_width):
    global LAST_RESULT
    assert int(attention_width) == WWIN
    x = np.ascontiguousarray(np.asarray(inputs, dtype=np.float32))
    t, b, h = x.shape
    assert b == B_FULL and h == H
    r = t * BL
    in_maps = make_in_maps(x, weight_W, weight_proj)
    nc = _get_nc(r)
    res = run_bass_kernel_spmd(
        nc, in_maps, core_ids=list(range(N_CORES)), trace=TRACE
    )
    LAST_RESULT = res
    out = np.empty((t, B_FULL, H), np.float32)
    for c, rmap in enumerate(res.results):
        out[:, BL * c:BL * (c + 1), :] = rmap["out"].reshape(t, BL, H)
    return out


# revision 15
# speedup vs baseline: 75706.9974x; 68909.9956x over previous
"""Trainium2 Bass kernel for nn_AttentionLayer (sliding-window attention).

Reference computation (per timestep t, batch b):
    scores = tanh(x @ W) @ proj                  # [T, B]
    for t >= w:  out[t] = sum_j softmax_j(scores[t-w .. t-1]) * x[t-w+j]
    for t <  w:  out[t] = x[t]
with T=2048, B=16, H=1024, w=3.

Strategy (8 NeuronCores, data-parallel over B — 2 batch columns per core):
  Per core, rows r = t*2 + beta (4096 rows of H=1024).  Row-shift of one
  timestep == shift of 2 rows, so out[r] = sum_j a_j[r] * x[r-6+2j].

  1. scores: y^T = W-stationary matmuls against host-pretransposed xT tiles
     (PE, float32r in / fp32 accumulate — single-pass, 4x the fp32 rate at
     ~11-bit multiply precision), tanh on ACT (float32r out), then
     s^T = proj^T @ tanh(y^T) on PE.  s lands as a [1, 4096] row in SBUF.
  2. softmax over the width-3 window: pure free-dim shifted views of s.
     One exp pass (ACT), esum + reciprocal + 3 normalized-weight writes
     (DVE, fp16 weights).
  3. weighted sum: one fp32 PE matmul per 122-row output tile:
     out = D^T @ x_tile where D is a [128, 122] fp32 matrix carrying the
     three a_j diagonals.  D is built for four tiles at a time: an fp16
     rank-1 PE broadcast of the weight rows into PSUM, then DVE
     copy_predicated with constant uint8 diagonal masks.  The 6-row halo
     is folded into the x-tile DMA, so no partition-shifted vector ops
     exist anywhere.

kernel() is self-contained: takes full inputs, shards over batch, runs
SPMD on cores 0..7, reassembles the full [T, B, H] output.
"""

import numpy as np
import ml_dtypes

import concourse.bacc as bacc
import concourse.mybir as mybir
import concourse.tile as tile
from concourse.bass_utils import run_bass_kernel_spmd

T_FULL, B_FULL, H = 2048, 16, 1024
N_CORES = 8
BL = B_FULL // N_CORES          # batch columns per core (2)
WWIN = 3                        # attention width (hardcoded)
HALO = WWIN * BL                # 6 rows of halo
MT = 128 - HALO                 # 122 output rows per weighted-sum tile
QT = 4                          # m-tiles per D-build quad group
ST = 512                        # score super-tile rows
KC = H // 128                   # 8 contraction chunks
F32 = mybir.dt.float32
F32R = mybir.dt.float32r
BF16 = mybir.dt.bfloat16
F16 = mybir.dt.float16
U8 = mybir.dt.uint8

TRACE = False                   # set True (from test.py) to capture an NTFF trace
LAST_RESULT = None              # BassKernelResults of the most recent run

# dtype knobs (all measured on HW):
#   SCORE_BF16: xT/W in bf16 (halves score DMA; PE same or faster)
#   WSUM_F16:   D and x in fp16 for the weighted sum (4x PE, ~1e-3 output err)
SCORE_BF16 = False
WSUM_F16 = True


def _mt_span(R, mt):
    m0 = mt * MT
    return m0, min(MT, R - m0)


def build_nc(R, loop_iters=0):
    """Build the single-core Bass program for R rows (R % 512 == 0).

    loop_iters > 0 wraps the whole computation in a hardware loop running it
    that many times — used only for device-time measurement (the per-call
    dispatch overhead through the axon tunnel dwarfs a single 0.2 ms run).
    """
    assert R % ST == 0
    nst = R // ST
    nmt = (R + MT - 1) // MT
    ngr = (nmt + QT - 1) // QT
    QW = QT * MT                 # weight columns per quad group (488)

    sd = BF16 if SCORE_BF16 else F32R     # score matmul input dtype
    wd = F16 if WSUM_F16 else F32         # weighted-sum matmul dtype

    nc = bacc.Bacc("TRN2", target_bir_lowering=False)

    x_d = nc.dram_tensor("x", [R, H], wd, kind="ExternalInput")
    xT_d = nc.dram_tensor("xT", [H, R], sd, kind="ExternalInput")
    w_d = nc.dram_tensor("w", [H, H], sd, kind="ExternalInput")
    proj_d = nc.dram_tensor("proj", [H], F32R, kind="ExternalInput")
    masksq_d = nc.dram_tensor("masksq", [128, 3 * QT * MT], U8, kind="ExternalInput")
    masksq0_d = nc.dram_tensor("masksq0", [128, 3 * QT * MT], U8, kind="ExternalInput")
    d0_d = nc.dram_tensor("d0init", [128, QT * MT], wd, kind="ExternalInput")
    xh_d = nc.dram_tensor("x_head", [8, H], F32, kind="ExternalInput")
    out_d = nc.dram_tensor("out", [R, H], F32, kind="ExternalOutput")
    QW = QT * MT

    with tile.TileContext(nc) as tc:
        with (
            tc.tile_pool(name="const", bufs=1) as const,
            tc.tile_pool(name="psum_y", bufs=2, space="PSUM") as psum_y,
            tc.tile_pool(name="psum_s", bufs=1, space="PSUM") as psum_s,
            tc.tile_pool(name="psum_bc", bufs=2, space="PSUM") as psum_bc,
            tc.tile_pool(name="psum_o", bufs=3, space="PSUM") as psum_o,
            tc.tile_pool(name="xt", bufs=12) as xt_pool,
            tc.tile_pool(name="th", bufs=4) as th_pool,
            tc.tile_pool(name="xn", bufs=3) as xn_pool,
            tc.tile_pool(name="ob", bufs=3) as ob_pool,
            tc.tile_pool(name="sm", bufs=2) as sm_pool,
        ):
            # ---- constants / persistent buffers ----
            w_sb = const.tile([128, KC * H], sd, name="w_sb")
            nc.sync.dma_start(
                w_sb[:, :].rearrange("p (kc h) -> p kc h", kc=KC),
                w_d.rearrange("(kc p) h -> p kc h", p=128),
            )
            proj_sb = const.tile([128, KC], F32R, name="proj_sb")
            nc.sync.dma_start(proj_sb[:, :], proj_d.rearrange("(c p) -> p c", p=128))
            masksq_sb = const.tile([128, 3 * QW], U8, name="masksq_sb")
            nc.sync.dma_start(masksq_sb[:, :], masksq_d[:, :])
            masksq0_sb = const.tile([128, 3 * QW], U8, name="masksq0_sb")
            nc.sync.dma_start(masksq0_sb[:, :], masksq0_d[:, :])
            d0t = const.tile([128, QW], wd, name="d0t")
            nc.sync.dma_start(d0t[:, :], d0_d[:, :])
            dA = const.tile([128, QW], wd, name="dA")
            dB = const.tile([128, QW], wd, name="dB")
            nc.vector.memset(dA[:, :], 0.0)
            nc.vector.memset(dB[:, :], 0.0)
            ones_sb = const.tile([1, 128], F16, name="ones_sb")
            nc.vector.memset(ones_sb[:, :], 1.0)
            # s_buf[0, 6 + r] = scores[r]; zero prefix covers the r < 6 window
            s_buf = const.tile([1, HALO + R + 64], F32, name="s_buf")
            nc.vector.memset(s_buf[:, :], 0.0)
            # e_all[0, i] = exp(s_buf[0, i]); a_j[0, r] = softmax weight (fp16)
            e_all = const.tile([1, R + 8], F32, name="e_all")
            a_bufs = [const.tile([1, R], F16, name=f"a{j}") for j in range(WWIN)]

            def emit_score(st):
                r0 = st * ST
                xts = []
                for kc in range(KC):
                    xt_t = xt_pool.tile([128, ST], sd, name="xt_t", tag="xt")
                    nc.sync.dma_start(
                        xt_t[:, :], xT_d[kc * 128:(kc + 1) * 128, r0:r0 + ST]
                    )
                    xts.append(xt_t)
                ths = []
                for ho in range(KC):
                    ypsum = psum_y.tile([128, ST], F32, name="ypsum", tag="ypsum")
                    for kc in range(KC):
                        off = kc * H + ho * 128
                        nc.tensor.matmul(
                            ypsum[:, :],
                            lhsT=w_sb[:, off:off + 128],
                            rhs=xts[kc][:, :],
                            start=(kc == 0),
                            stop=(kc == KC - 1),
                        )
                    th = th_pool.tile([128, ST], F32R, name="th", tag="th")
                    nc.scalar.activation(
                        th[:, :], ypsum[:, :], mybir.ActivationFunctionType.Tanh
                    )
                    ths.append(th)
                spsum = psum_s.tile([1, ST], F32, name="spsum", tag="spsum")
                for ho in range(KC):
                    nc.tensor.matmul(
                        spsum[:, :],
                        lhsT=proj_sb[:, ho:ho + 1],
                        rhs=ths[ho][:, :],
                        start=(ho == 0),
                        stop=(ho == KC - 1),
                    )
                nc.scalar.copy(s_buf[0:1, HALO + r0:HALO + r0 + ST], spsum[:, :])
                # one exp pass over this chunk of s (overlapping 4 into the
                # next chunk so the window sums below never read ahead)
                en = min(ST + 4, R + 4 - r0)
                nc.scalar.activation(
                    e_all[0:1, r0:r0 + en],
                    s_buf[0:1, r0:r0 + en],
                    mybir.ActivationFunctionType.Exp,
                )
                esum = sm_pool.tile([1, ST], F32, name="esum", tag="esum")
                nc.vector.tensor_add(
                    esum[:, :], e_all[0:1, r0:r0 + ST], e_all[0:1, r0 + 2:r0 + 2 + ST]
                )
                nc.vector.tensor_add(
                    esum[:, :], esum[:, :], e_all[0:1, r0 + 4:r0 + 4 + ST]
                )
                nc.vector.reciprocal(esum[:, :], esum[:, :])
                for j in range(WWIN):
                    nc.vector.tensor_mul(
                        a_bufs[j][0:1, r0:r0 + ST],
                        e_all[0:1, r0 + 2 * j:r0 + 2 * j + ST],
                        esum[:, :],
                    )

            def emit_group(g):
                """Build the quad D matrix for m-tiles [QT*g, QT*g+QT) and run
                their weighted-sum matmuls."""
                mts = list(range(QT * g, min(QT * g + QT, nmt)))
                g0 = QT * g * MT                            # first weight column
                gn = sum(_mt_span(R, mt)[1] for mt in mts)  # total weight cols
                if g == 0:
                    dmat, msk = d0t, masksq0_sb
                else:
                    dmat = dA if (g % 2) else dB
                    msk = masksq_sb
                for j in range(WWIN):
                    bcast = psum_bc.tile([128, QW], F32, name="bcast", tag="bc")
                    nc.tensor.matmul(
                        bcast[:, 0:gn],
                        lhsT=ones_sb[:, :],
                        rhs=a_bufs[j][0:1, g0:g0 + gn],
                        start=True,
                        stop=True,
                    )
                    nc.vector.copy_predicated(
                        dmat[:, 0:gn], msk[:, j * QW:j * QW + gn], bcast[:, 0:gn]
                    )
                for qi, mt in enumerate(mts):
                    m0, m_n = _mt_span(R, mt)
                    if mt == 0:
                        src0, k_n = 0, 128
                    else:
                        src0 = m0 - HALO
                        k_n = min(128, R - src0)
                    xn = xn_pool.tile([128, H], wd, name="xn", tag="xn")
                    nc.sync.dma_start(xn[0:k_n, :], x_d[src0:src0 + k_n, :])
                    ob = ob_pool.tile([MT, H], F32, name="ob", tag="ob")
                    for half in range(2):
                        op_ = psum_o.tile([MT, 512], F32, name="op_", tag="opsum")
                        nc.tensor.matmul(
                            op_[0:m_n, :],
                            lhsT=dmat[0:k_n, qi * MT:qi * MT + m_n],
                            rhs=xn[0:k_n, half * 512:(half + 1) * 512],
                            start=True,
                            stop=True,
                        )
                        if half == 0:
                            nc.vector.tensor_copy(ob[0:m_n, 0:512], op_[0:m_n, :])
                        else:
                            nc.scalar.copy(ob[0:m_n, 512:1024], op_[0:m_n, :])
                    if mt == 0:
                        # exact fp32 passthrough for rows with t < w
                        nc.sync.dma_start(ob[0:HALO, :], xh_d[0:HALO, :])
                    nc.sync.dma_start(out_d[m0:m0 + m_n, :], ob[0:m_n, :])

            # software-pipelined emission: a quad group is emitted as soon as
            # the score super-tile covering its last row has been emitted
            emit_at = {st: [] for st in range(nst)}
            for g in range(ngr):
                last_mt = min(QT * g + QT, nmt) - 1
                last_row = min(last_mt * MT + _mt_span(R, last_mt)[1] - 1, R - 1)
                emit_at[min(last_row // ST, nst - 1)].append(g)
            def emit_all():
                for st in range(nst):
                    emit_score(st)
                    for g in emit_at[st]:
                        emit_group(g)

            emitted_consts = []
            if loop_iters:
                with tc.For_i(0, loop_iters, 1):
                    emit_all()
            else:
                emit_all()

    nc.compile()
    return nc


def make_consts(R):
    QW = QT * MT
    masksq = np.zeros((128, 3 * QW), np.uint8)
    masksq0 = np.zeros((128, 3 * QW), np.uint8)
    d0 = np.zeros((128, QW), np.float32)
    for j in range(WWIN):
        for q in range(QT):
            for m in range(MT):
                col = j * QW + q * MT + m
                masksq[m + 2 * j, col] = 1
                if q == 0:
                    if m >= HALO:
                        masksq0[m - HALO + 2 * j, col] = 1
                else:
                    masksq0[m + 2 * j, col] = 1
    for m in range(HALO):
        d0[m, m] = 1.0
    return masksq, masksq0, d0


_NC_CACHE = {}


def _get_nc(R):
    key = (R, SCORE_BF16, WSUM_F16)
    if key not in _NC_CACHE:
        _NC_CACHE[key] = build_nc(R)
    return _NC_CACHE[key]


def make_in_maps(x, weight_W, weight_proj):
    """x: [T, B, H] fp32 -> list of per-core input dicts."""
    sd = ml_dtypes.bfloat16 if SCORE_BF16 else np.float32
    wd = np.float16 if WSUM_F16 else np.float32
    W = np.ascontiguousarray(np.asarray(weight_W, dtype=np.float32)).astype(sd)
    proj = np.ascontiguousarray(
        np.asarray(weight_proj, dtype=np.float32).reshape(H)
    )
    t = x.shape[0]
    masksq, masksq0, d0 = make_consts(t * BL)
    d0 = d0.astype(wd)
    in_maps = []
    for c in range(N_CORES):
        xc = np.ascontiguousarray(x[:, BL * c:BL * (c + 1), :]).reshape(t * BL, H)
        in_maps.append(
            dict(
                x_head=np.ascontiguousarray(xc[0:8]),
                x=xc.astype(wd) if wd is not np.float32 else xc,
                xT=np.ascontiguousarray(xc.T).astype(sd),
                w=W,
                proj=proj,
                masksq=masksq,
                masksq0=masksq0,
                d0init=d0,
            )
        )
    return in_maps


def kernel(inputs, weight_W, weight_proj, attention_width):
    global LAST_RESULT
    assert int(attention_width) == WWIN
    x = np.ascontiguousarray(np.asarray(inputs, dtype=np.float32))
    t, b, h = x.shape
    assert b == B_FULL and h == H
    r = t * BL
    in_maps = make_in_maps(x, weight_W, weight_proj)
    nc = _get_nc(r)
    res = run_bass_kernel_spmd(
        nc, in_maps, core_ids=list(range(N_CORES)), trace=TRACE
    )
    LAST_RESULT = res
    out = np.empty((t, B_FULL, H), np.float32)
    for c, rmap in enumerate(res.results):
        out[:, BL * c:BL * (c + 1), :] = rmap["out"].reshape(t, BL, H)
    return out


# revision 17
# speedup vs baseline: 86067.7372x; 1.1369x over previous
"""Trainium2 Bass kernel for nn_AttentionLayer (sliding-window attention).

Reference computation (per timestep t, batch b):
    scores = tanh(x @ W) @ proj                  # [T, B]
    for t >= w:  out[t] = sum_j softmax_j(scores[t-w .. t-1]) * x[t-w+j]
    for t <  w:  out[t] = x[t]
with T=2048, B=16, H=1024, w=3.

Strategy (8 NeuronCores, data-parallel over B — 2 batch columns per core):
  Per core, rows r = t*2 + beta (4096 rows of H=1024).  Row-shift of one
  timestep == shift of 2 rows, so out[r] = sum_j a_j[r] * x[r-6+2j].

  1. scores: y^T = W-stationary matmuls against host-pretransposed xT tiles
     (PE, float32r in / fp32 accumulate — single-pass, 4x the fp32 rate at
     ~11-bit multiply precision), tanh on ACT (float32r out), then
     s^T = proj^T @ tanh(y^T) on PE.  s lands as a [1, 4096] row in SBUF.
  2. softmax over the width-3 window: pure free-dim shifted views of s.
     One exp pass (ACT), esum + reciprocal + 3 normalized-weight writes
     (DVE, fp16 weights).
  3. weighted sum: one fp32 PE matmul per 122-row output tile:
     out = D^T @ x_tile where D is a [128, 122] fp32 matrix carrying the
     three a_j diagonals.  D is built for four tiles at a time: an fp16
     rank-1 PE broadcast of the weight rows into PSUM, then DVE
     copy_predicated with constant uint8 diagonal masks.  The 6-row halo
     is folded into the x-tile DMA, so no partition-shifted vector ops
     exist anywhere.

kernel() is self-contained: takes full inputs, shards over batch, runs
SPMD on cores 0..7, reassembles the full [T, B, H] output.
"""

import numpy as np
import ml_dtypes

import concourse.bacc as bacc
import concourse.mybir as mybir
import concourse.tile as tile
from concourse.bass_utils import run_bass_kernel_spmd

T_FULL, B_FULL, H = 2048, 16, 1024
N_CORES = 8
BL = B_FULL // N_CORES          # batch columns per core (2)
WWIN = 3                        # attention width (hardcoded)
HALO = WWIN * BL                # 6 rows of halo
MT = 128 - HALO                 # 122 output rows per weighted-sum tile
QT = 4                          # m-tiles per D-build quad group
ST = 512                        # score super-tile rows
KC = H // 128                   # 8 contraction chunks
F32 = mybir.dt.float32
F32R = mybir.dt.float32r
BF16 = mybir.dt.bfloat16
F16 = mybir.dt.float16
U8 = mybir.dt.uint8

TRACE = False                   # set True (from test.py) to capture an NTFF trace
LAST_RESULT = None              # BassKernelResults of the most recent run

# dtype knobs (all measured on HW):
#   SCORE_BF16: xT/W in bf16 (halves score DMA; PE same or faster)
#   WSUM_F16:   D and x in fp16 for the weighted sum (4x PE, ~1e-3 output err)
SCORE_BF16 = False
WSUM_F16 = True


def _mt_span(R, mt):
    m0 = mt * MT
    return m0, min(MT, R - m0)


def build_nc(R, loop_iters=0):
    """Build the single-core Bass program for R rows (R % 512 == 0).

    loop_iters > 0 wraps the whole computation in a hardware loop running it
    that many times — used only for device-time measurement (the per-call
    dispatch overhead through the axon tunnel dwarfs a single 0.2 ms run).
    """
    assert R % ST == 0
    nst = R // ST
    nmt = (R + MT - 1) // MT
    ngr = (nmt + QT - 1) // QT
    QW = QT * MT                 # weight columns per quad group (488)

    sd = BF16 if SCORE_BF16 else F32R     # score matmul input dtype
    wd = F16 if WSUM_F16 else F32         # weighted-sum matmul dtype

    nc = bacc.Bacc("TRN2", target_bir_lowering=False)

    x_d = nc.dram_tensor("x", [R, H], wd, kind="ExternalInput")
    xT_d = nc.dram_tensor("xT", [H, R], sd, kind="ExternalInput")
    w_d = nc.dram_tensor("w", [H, H], sd, kind="ExternalInput")
    proj_d = nc.dram_tensor("proj", [H], F32R, kind="ExternalInput")
    masksq_d = nc.dram_tensor("masksq", [128, 3 * QT * MT], U8, kind="ExternalInput")
    masksq0_d = nc.dram_tensor("masksq0", [128, 3 * QT * MT], U8, kind="ExternalInput")
    d0_d = nc.dram_tensor("d0init", [128, QT * MT], wd, kind="ExternalInput")
    xh_d = nc.dram_tensor("x_head", [8, H], F32, kind="ExternalInput")
    out_d = nc.dram_tensor("out", [R, H], F32, kind="ExternalOutput")
    QW = QT * MT

    with tile.TileContext(nc) as tc:
        with (
            tc.tile_pool(name="const", bufs=1) as const,
            tc.tile_pool(name="psum_y", bufs=2, space="PSUM") as psum_y,
            tc.tile_pool(name="psum_s", bufs=1, space="PSUM") as psum_s,
            tc.tile_pool(name="psum_bc", bufs=2, space="PSUM") as psum_bc,
            tc.tile_pool(name="psum_o", bufs=3, space="PSUM") as psum_o,
            tc.tile_pool(name="xt", bufs=16) as xt_pool,
            tc.tile_pool(name="th", bufs=6) as th_pool,
            tc.tile_pool(name="xn", bufs=6) as xn_pool,
            tc.tile_pool(name="ob", bufs=4) as ob_pool,
            tc.tile_pool(name="sm", bufs=2) as sm_pool,
        ):
            # ---- constants / persistent buffers ----
            w_sb = const.tile([128, KC * H], sd, name="w_sb")

            def emit_w_dma(ho):
                nc.sync.dma_start(
                    w_sb[:, :].rearrange("p (kc h) -> p kc h", kc=KC)[
                        :, :, ho * 128:(ho + 1) * 128
                    ],
                    w_d.rearrange("(kc p) h -> p kc h", p=128)[
                        :, :, ho * 128:(ho + 1) * 128
                    ],
                )
            proj_sb = const.tile([128, KC], F32R, name="proj_sb")
            nc.sync.dma_start(proj_sb[:, :], proj_d.rearrange("(c p) -> p c", p=128))
            masksq_sb = const.tile([128, 3 * QW], U8, name="masksq_sb")
            masksq0_sb = const.tile([128, 3 * QW], U8, name="masksq0_sb")
            d0t = const.tile([128, QW], wd, name="d0t")

            def emit_const_dmas():
                nc.sync.dma_start(masksq_sb[:, :], masksq_d[:, :])
                nc.sync.dma_start(masksq0_sb[:, :], masksq0_d[:, :])
                nc.sync.dma_start(d0t[:, :], d0_d[:, :])
            dA = const.tile([128, QW], wd, name="dA")
            dB = const.tile([128, QW], wd, name="dB")
            nc.vector.memset(dA[:, :], 0.0)
            nc.vector.memset(dB[:, :], 0.0)
            ones_sb = const.tile([1, 128], F16, name="ones_sb")
            nc.vector.memset(ones_sb[:, :], 1.0)
            # s_buf[0, 6 + r] = scores[r]; zero prefix covers the r < 6 window
            s_buf = const.tile([1, HALO + R + 64], F32, name="s_buf")
            nc.vector.memset(s_buf[:, :], 0.0)
            # e_all[0, i] = exp(s_buf[0, i]); a_j[0, r] = softmax weight (fp16)
            e_all = const.tile([1, R + 8], F32, name="e_all")
            a_bufs = [const.tile([1, R], F16, name=f"a{j}") for j in range(WWIN)]

            def emit_score(st):
                r0 = st * ST
                xts = []
                for kc in range(KC):
                    xt_t = xt_pool.tile([128, ST], sd, name="xt_t", tag="xt")
                    nc.sync.dma_start(
                        xt_t[:, :], xT_d[kc * 128:(kc + 1) * 128, r0:r0 + ST]
                    )
                    xts.append(xt_t)
                    if st == 0:
                        emit_w_dma(kc)
                ths = []
                for ho in range(KC):
                    ypsum = psum_y.tile([128, ST], F32, name="ypsum", tag="ypsum")
                    for kc in range(KC):
                        off = kc * H + ho * 128
                        nc.tensor.matmul(
                            ypsum[:, :],
                            lhsT=w_sb[:, off:off + 128],
                            rhs=xts[kc][:, :],
                            start=(kc == 0),
                            stop=(kc == KC - 1),
                        )
                    th = th_pool.tile([128, ST], F32R, name="th", tag="th")
                    nc.scalar.activation(
                        th[:, :], ypsum[:, :], mybir.ActivationFunctionType.Tanh
                    )
                    ths.append(th)
                spsum = psum_s.tile([1, ST], F32, name="spsum", tag="spsum")
                for ho in range(KC):
                    nc.tensor.matmul(
                        spsum[:, :],
                        lhsT=proj_sb[:, ho:ho + 1],
                        rhs=ths[ho][:, :],
                        start=(ho == 0),
                        stop=(ho == KC - 1),
                    )
                nc.scalar.copy(s_buf[0:1, HALO + r0:HALO + r0 + ST], spsum[:, :])
                # one exp pass over this chunk of s (overlapping 4 into the
                # next chunk so the window sums below never read ahead)
                en = min(ST + 4, R + 4 - r0)
                nc.scalar.activation(
                    e_all[0:1, r0:r0 + en],
                    s_buf[0:1, r0:r0 + en],
                    mybir.ActivationFunctionType.Exp,
                )
                esum = sm_pool.tile([1, ST], F32, name="esum", tag="esum")
                nc.vector.tensor_add(
                    esum[:, :], e_all[0:1, r0:r0 + ST], e_all[0:1, r0 + 2:r0 + 2 + ST]
                )
                nc.vector.tensor_add(
                    esum[:, :], esum[:, :], e_all[0:1, r0 + 4:r0 + 4 + ST]
                )
                nc.vector.reciprocal(esum[:, :], esum[:, :])
                for j in range(WWIN):
                    nc.vector.tensor_mul(
                        a_bufs[j][0:1, r0:r0 + ST],
                        e_all[0:1, r0 + 2 * j:r0 + 2 * j + ST],
                        esum[:, :],
                    )

            def emit_group(g):
                """Build the quad D matrix for m-tiles [QT*g, QT*g+QT) and run
                their weighted-sum matmuls."""
                mts = list(range(QT * g, min(QT * g + QT, nmt)))
                g0 = QT * g * MT                            # first weight column
                gn = sum(_mt_span(R, mt)[1] for mt in mts)  # total weight cols
                if g == 0:
                    dmat, msk = d0t, masksq0_sb
                else:
                    dmat = dA if (g % 2) else dB
                    msk = masksq_sb
                for j in range(WWIN):
                    bcast = psum_bc.tile([128, QW], F32, name="bcast", tag="bc")
                    nc.tensor.matmul(
                        bcast[:, 0:gn],
                        lhsT=ones_sb[:, :],
                        rhs=a_bufs[j][0:1, g0:g0 + gn],
                        start=True,
                        stop=True,
                    )
                    nc.vector.copy_predicated(
                        dmat[:, 0:gn], msk[:, j * QW:j * QW + gn], bcast[:, 0:gn]
                    )
                for qi, mt in enumerate(mts):
                    m0, m_n = _mt_span(R, mt)
                    if mt == 0:
                        src0, k_n = 0, 128
                    else:
                        src0 = m0 - HALO
                        k_n = min(128, R - src0)
                    xn = xn_pool.tile([128, H], wd, name="xn", tag="xn")
                    nc.sync.dma_start(xn[0:k_n, :], x_d[src0:src0 + k_n, :])
                    ob = ob_pool.tile([MT, H], F32, name="ob", tag="ob")
                    for half in range(2):
                        op_ = psum_o.tile([MT, 512], F32, name="op_", tag="opsum")
                        nc.tensor.matmul(
                            op_[0:m_n, :],
                            lhsT=dmat[0:k_n, qi * MT:qi * MT + m_n],
                            rhs=xn[0:k_n, half * 512:(half + 1) * 512],
                            start=True,
                            stop=True,
                        )
                        if half == 0:
                            nc.vector.tensor_copy(ob[0:m_n, 0:512], op_[0:m_n, :])
                        else:
                            nc.scalar.copy(ob[0:m_n, 512:1024], op_[0:m_n, :])
                    if mt == 0:
                        # exact fp32 passthrough for rows with t < w
                        nc.sync.dma_start(ob[0:HALO, :], xh_d[0:HALO, :])
                    nc.sync.dma_start(out_d[m0:m0 + m_n, :], ob[0:m_n, :])

            # software-pipelined emission: a quad group is emitted as soon as
            # the score super-tile covering its last row has been emitted
            emit_at = {st: [] for st in range(nst)}
            for g in range(ngr):
                last_mt = min(QT * g + QT, nmt) - 1
                last_row = min(last_mt * MT + _mt_span(R, last_mt)[1] - 1, R - 1)
                emit_at[min(last_row // ST, nst - 1)].append(g)
            def emit_all():
                for st in range(nst):
                    emit_score(st)
                    if st == 0:
                        emit_const_dmas()
                    for g in emit_at[st]:
                        emit_group(g)

            emitted_consts = []
            if loop_iters:
                with tc.For_i(0, loop_iters, 1):
                    emit_all()
            else:
                emit_all()

    nc.compile()
    return nc


def make_consts(R):
    QW = QT * MT
    masksq = np.zeros((128, 3 * QW), np.uint8)
    masksq0 = np.zeros((128, 3 * QW), np.uint8)
    d0 = np.zeros((128, QW), np.float32)
    for j in range(WWIN):
        for q in range(QT):
            for m in range(MT):
                col = j * QW + q * MT + m
                masksq[m + 2 * j, col] = 1
                if q == 0:
                    if m >= HALO:
                        masksq0[m - HALO + 2 * j, col] = 1
                else:
                    masksq0[m + 2 * j, col] = 1
    for m in range(HALO):
        d0[m, m] = 1.0
    return masksq, masksq0, d0


_NC_CACHE = {}


def _get_nc(R):
    key = (R, SCORE_BF16, WSUM_F16)
    if key not in _NC_CACHE:
        _NC_CACHE[key] = build_nc(R)
    return _NC_CACHE[key]


def make_in_maps(x, weight_W, weight_proj):
    """x: [T, B, H] fp32 -> list of per-core input dicts."""
    sd = ml_dtypes.bfloat16 if SCORE_BF16 else np.float32
    wd = np.float16 if WSUM_F16 else np.float32
    W = np.ascontiguousarray(np.asarray(weight_W, dtype=np.float32)).astype(sd)
    proj = np.ascontiguousarray(
        np.asarray(weight_proj, dtype=np.float32).reshape(H)
    )
    t = x.shape[0]
    masksq, masksq0, d0 = make_consts(t * BL)
    d0 = d0.astype(wd)
    in_maps = []
    for c in range(N_CORES):
        xc = np.ascontiguousarray(x[:, BL * c:BL * (c + 1), :]).reshape(t * BL, H)
        in_maps.append(
            dict(
                x_head=np.ascontiguousarray(xc[0:8]),
                x=xc.astype(wd) if wd is not np.float32 else xc,
                xT=np.ascontiguousarray(xc.T).astype(sd),
                w=W,
                proj=proj,
                masksq=masksq,
                masksq0=masksq0,
                d0init=d0,
            )
        )
    return in_maps


def kernel(inputs, weight_W, weight_proj, attention_width):
    global LAST_RESULT
    assert int(attention_width) == WWIN
    x = np.ascontiguousarray(np.asarray(inputs, dtype=np.float32))
    t, b, h = x.shape
    assert b == B_FULL and h == H
    r = t * BL
    in_maps = make_in_maps(x, weight_W, weight_proj)
    nc = _get_nc(r)
    res = run_bass_kernel_spmd(
        nc, in_maps, core_ids=list(range(N_CORES)), trace=TRACE
    )
    LAST_RESULT = res
    out = np.empty((t, B_FULL, H), np.float32)
    for c, rmap in enumerate(res.results):
        out[:, BL * c:BL * (c + 1), :] = rmap["out"].reshape(t, BL, H)
    return out
